# revision 22
# baseline (speedup 1.0000x reference)
"""Trainium2 Bass kernel for nn_CompositeK (retrieval_knn).

Self-contained: only imports from /opt/trn_rl_repo (system install) + numpy.

Sharding: 8 cores; core c handles batch b=c//2, query-half h=c%2 (2048 rows).
Each core receives x for its batch ROTATED so its own rows are tokens 0..2047,
pre-transposed to xT [1024, 4096].  All device outputs are in [feature, token]
layout; the host transposes back and un-rotates kNN indices.

Per core:
  phase 1: embT (fp32, all 4096 tokens) + norms (ones-matmul) -> normalized embT
  phase 2: sim = embnT^T @ embnT per 128-query tile (fp32), diagonal masked,
           exact top-32 via per-128-block max8 screening + max8/match_replace
           merge + max_index for global indices
  phase 3: metric/christoffel/ecc/coords/K_flat as float32r matmuls (1 cyc/row)

DMA instructions are batched (3D-tile weight loads, whole-row output staging)
and issue alternates between the SP and ACT HWDGE queues: DMA *issue* costs
~1.2us of sequencer time each, which was the original bottleneck.
"""

import os
import sys

import numpy as np

sys.path.insert(0, "/opt/trn_rl_repo")

import concourse.tile as tile  # noqa: E402
from concourse import bacc  # noqa: E402
from concourse import mybir  # noqa: E402
from concourse.bass_utils import run_bass_kernel_spmd  # noqa: E402

F32 = mybir.dt.float32
F32R = mybir.dt.float32r
U16 = mybir.dt.uint16
I16 = mybir.dt.int16
I32 = mybir.dt.int32
AF = mybir.ActivationFunctionType
OP = mybir.AluOpType

P = 128
S = 4096
HALF = 2048
D = 1024
DE = 256
NCH = 32
ECC = 32
DC = 256
KF = 512
K = 32
NEG = -1.0e9

LAST_RESULTS = None
LAST_IN_MAPS = None


def build_nc():
    nc = bacc.Bacc("TRN2", target_bir_lowering=False, debug=False, num_devices=8)

    # ---------------- I/O ----------------
    xt_d = nc.dram_tensor("xt", [D, S], F32, kind="ExternalInput")
    xtr_d = nc.dram_tensor("xtr", [D, HALF], F32R, kind="ExternalInput")
    w_embed_d = nc.dram_tensor("w_embed", [D, DE], F32, kind="ExternalInput")
    w_diag_d = nc.dram_tensor("w_diag", [D, D], F32R, kind="ExternalInput")
    w_chr_d = nc.dram_tensor("w_chr", [D, NCH], F32R, kind="ExternalInput")
    w_eccp_d = nc.dram_tensor("w_eccp", [D, ECC], F32R, kind="ExternalInput")
    w_ecc1_d = nc.dram_tensor("w_ecc1", [ECC, 2 * ECC], F32R, kind="ExternalInput")
    w_ecc2_d = nc.dram_tensor("w_ecc2", [2 * ECC, ECC], F32R, kind="ExternalInput")
    w_coord_d = nc.dram_tensor("w_coord", [D, DC], F32R, kind="ExternalInput")
    w_kout_d = nc.dram_tensor("w_kout", [KF, D], F32R, kind="ExternalInput")
    biases_d = nc.dram_tensor("biases", [P, 16], F32, kind="ExternalInput")
    ident_d = nc.dram_tensor("ident", [P, P], F32, kind="ExternalInput")
    segoff_d = nc.dram_tensor("segoff", [P, 256], U16, kind="ExternalInput")
    rankv_d = nc.dram_tensor("rankv", [P, K], I16, kind="ExternalInput")

    o_embT = nc.dram_tensor("o_embT", [DE, HALF], F32, kind="ExternalOutput")
    o_metT = nc.dram_tensor("o_metT", [D, HALF], F32, kind="ExternalOutput")
    o_chrT = nc.dram_tensor("o_chrT", [NCH, HALF], F32, kind="ExternalOutput")
    o_scores = nc.dram_tensor("o_scores", [HALF, K], F32, kind="ExternalOutput")
    o_idx = nc.dram_tensor("o_idx", [HALF, K], I32, kind="ExternalOutput")
    o_eccT = nc.dram_tensor("o_eccT", [ECC, HALF], F32, kind="ExternalOutput")
    o_cooT = nc.dram_tensor("o_cooT", [DC, HALF], F32, kind="ExternalOutput")
    o_kflT = nc.dram_tensor("o_kflT", [D, HALF], F32, kind="ExternalOutput")

    with tile.TileContext(nc) as tc:
        pers = tc.alloc_tile_pool(name="pers", bufs=1)
        psum = tc.alloc_tile_pool(name="psum", bufs=6, space="PSUM")
        psum1 = tc.alloc_tile_pool(name="psum1", bufs=2, space="PSUM")

        # alternate DMA issue between the SP and ACT HWDGE queues
        _dmai = [0]

        def dma(out, in_):
            eng = (nc.sync, nc.scalar)[_dmai[0] % 2]
            _dmai[0] += 1
            eng.dma_start(out, in_)

        # ------------- persistent loads (batched) -------------
        # xtr: [128, 8, 2048] f32r, 4 DMAs of 512-col stripes
        xtr_sb = pers.tile([P, 8, HALF], F32R, tag="xtr", name="xtr")
        xtr_r = xtr_d.rearrange("(ko p) t -> p ko t", p=P)
        for j in range(4):
            dma(xtr_sb[:, :, j * 512 : (j + 1) * 512], xtr_r[:, :, j * 512 : (j + 1) * 512])

        w_embed_sb = pers.tile([P, 8, DE], F32, tag="wemb", name="wemb")
        dma(w_embed_sb[:], w_embed_d.rearrange("(ko p) n -> p ko n", p=P))
        w_chr_sb = pers.tile([P, 8, NCH], F32R, tag="wchr", name="wchr")
        dma(w_chr_sb[:], w_chr_d.rearrange("(ko p) n -> p ko n", p=P))
        w_eccp_sb = pers.tile([P, 8, ECC], F32R, tag="weccp", name="weccp")
        dma(w_eccp_sb[:], w_eccp_d.rearrange("(ko p) n -> p ko n", p=P))
        w_coord_sb = pers.tile([P, 8, DC], F32R, tag="wcoo", name="wcoo")
        dma(w_coord_sb[:], w_coord_d.rearrange("(ko p) n -> p ko n", p=P))
        w_kout_sb = pers.tile([P, 4, D], F32R, tag="wko", name="wko")
        dma(w_kout_sb[:], w_kout_d.rearrange("(ko p) n -> p ko n", p=P))
        w1p = pers.tile([ECC, 2 * ECC], F32R, tag="w1p", name="w1p")
        dma(w1p[:], w_ecc1_d[:])
        w2p = pers.tile([2 * ECC, ECC], F32R, tag="w2p", name="w2p")
        dma(w2p[:], w_ecc2_d[:])

        # biases packed [128, 16]: 0-1 b_embed, 2 b_chr, 3 b_eccp, 4 b_ecc1,
        # 5 b_ecc2, 6-7 b_coord, 8-15 b_kout
        bias_sb = pers.tile([P, 16], F32, tag="bias", name="bias")
        dma(bias_sb[:], biases_d[:])
        b_embed = lambda dt: bias_sb[:, dt : dt + 1]  # noqa: E731
        b_chr = bias_sb[0:NCH, 2:3]
        b_eccp = bias_sb[0:ECC, 3:4]
        b_ecc1 = bias_sb[0 : 2 * ECC, 4:5]
        b_ecc2 = bias_sb[0:ECC, 5:6]
        b_coord = lambda dt: bias_sb[:, 6 + dt : 7 + dt]  # noqa: E731
        b_kout = lambda nt: bias_sb[:, 8 + nt : 9 + nt]  # noqa: E731

        ident_sb = pers.tile([P, P], F32, tag="ident", name="ident")
        dma(ident_sb[:], ident_d[:])
        segoff_sb = pers.tile([P, 256], U16, tag="segoff", name="segoff")
        dma(segoff_sb[:], segoff_d[:])
        rankv_sb = pers.tile([P, K], I16, tag="rankv", name="rankv")
        dma(rankv_sb[:], rankv_d[:])

        embn_sb = [
            pers.tile([P, S], F32, tag=f"embn{dt}", name=f"embn{dt}") for dt in range(2)
        ]
        den_own = pers.tile([P, HALF], F32, tag="denown", name="denown")
        score_acc = pers.tile([P, 16, K], F32, tag="scacc", name="scacc")
        idx_acc = pers.tile([P, 16, K], I32, tag="idxacc", name="idxacc")

        # ============ Phase 3a: metric/chr/ecc/coords (xtr-only, runs early) ====
        N_MET = int(os.environ.get("KERNEL_N_MET", "8"))
        N_CH = int(os.environ.get("KERNEL_N_CH", "4"))
        with tc.tile_pool(name="ph3a", bufs=2) as ph3, tc.tile_pool(
            name="ph3aw", bufs=2
        ) as ph3w:
            wd_r = w_diag_d.rearrange("(ko p) n -> p ko n", p=P)
            for nt in range(N_MET):
                wcol = ph3w.tile([P, 8, P], F32R, tag="wd", name="wd")
                dma(wcol[:], wd_r[:, :, nt * P : (nt + 1) * P])
                st = ph3.tile([P, HALF], F32, tag="mst", name="mst")
                for c in range(4):
                    ps = psum.tile([P, 512], F32, tag="mm", name="mm")
                    for kt in range(8):
                        nc.tensor.matmul(
                            ps[:],
                            wcol[:, kt, :],
                            xtr_sb[:, kt, c * 512 : (c + 1) * 512],
                            start=(kt == 0),
                            stop=(kt == 7),
                        )
                    nc.scalar.activation(st[:, c * 512 : (c + 1) * 512], ps[:], AF.Copy)
                dma(o_metT[nt * P : (nt + 1) * P, :], st[:])

            for c in range(N_CH):
                cs = slice(c * 512, (c + 1) * 512)
                # christoffel
                ps = psum.tile([P, 512], F32, tag="mm", name="mm")
                for kt in range(8):
                    nc.tensor.matmul(
                        ps[0:NCH, :],
                        w_chr_sb[:, kt, :],
                        xtr_sb[:, kt, cs],
                        start=(kt == 0),
                        stop=(kt == 7),
                    )
                ch_st = ph3.tile([NCH, 512], F32, tag="chst", name="chst")
                nc.scalar.activation(ch_st[:], ps[0:NCH, :], AF.Identity, bias=b_chr)
                dma(o_chrT[:, cs], ch_st[:])
                # ecc chain
                ps = psum.tile([P, 512], F32, tag="mm", name="mm")
                for kt in range(8):
                    nc.tensor.matmul(
                        ps[0:ECC, :],
                        w_eccp_sb[:, kt, :],
                        xtr_sb[:, kt, cs],
                        start=(kt == 0),
                        stop=(kt == 7),
                    )
                pt = ph3.tile([ECC, 512], F32R, tag="pt", name="pt")
                nc.scalar.activation(pt[:], ps[0:ECC, :], AF.Identity, bias=b_eccp)
                ps = psum.tile([P, 512], F32, tag="mm", name="mm")
                nc.tensor.matmul(ps[0 : 2 * ECC, :], w1p[:], pt[:], start=True, stop=True)
                ht = ph3.tile([2 * ECC, 512], F32R, tag="ht", name="ht")
                nc.scalar.activation(ht[:], ps[0 : 2 * ECC, :], AF.Tanh, bias=b_ecc1)
                ps = psum.tile([P, 512], F32, tag="mm", name="mm")
                nc.tensor.matmul(ps[0:ECC, :], w2p[:], ht[:], start=True, stop=True)
                ec_st = ph3.tile([ECC, 512], F32, tag="ecst", name="ecst")
                nc.scalar.activation(ec_st[:], ps[0:ECC, :], AF.Sigmoid, bias=b_ecc2)
                dma(o_eccT[:, cs], ec_st[:])
                # coords
                for dt in range(2):
                    ps = psum.tile([P, 512], F32, tag="mm", name="mm")
                    for kt in range(8):
                        nc.tensor.matmul(
                            ps[:],
                            w_coord_sb[:, kt, dt * P : (dt + 1) * P],
                            xtr_sb[:, kt, cs],
                            start=(kt == 0),
                            stop=(kt == 7),
                        )
                    ct = ph3.tile([P, 512], F32R, tag=f"coo{dt}", name=f"coo{dt}")
                    nc.scalar.activation(ct[:], ps[:], AF.Identity, bias=b_coord(dt))
                    dma(o_cooT[dt * P : (dt + 1) * P, cs], ct[:].bitcast(F32))

        # ================= Phase 1: embeddings (full batch) =================
        with tc.tile_pool(name="ph1", bufs=1) as ph1, tc.tile_pool(
            name="ph1s", bufs=2
        ) as ph1s:
            ones_sb = ph1.tile([P, P], F32, tag="ones", name="ones")
            nc.gpsimd.memset(ones_sb[:], 1.0)
            nrm = ph1.tile([P, S], F32, tag="nrm", name="nrm")
            xt_r = xt_d.rearrange("(ko p) t -> p ko t", p=P)

            for half in (1, 0):
              for g in range(2):
                xs = ph1.tile([P, 8, 1024], F32, tag="xs", name="xs")
                for j in range(2):
                    c0 = half * HALF + g * 1024 + j * 512
                    dma(xs[:, :, j * 512 : (j + 1) * 512], xt_r[:, :, c0 : c0 + 512])
                for c in range(2):
                    col0 = half * HALF + g * 1024 + c * 512
                    loc = slice(c * 512, (c + 1) * 512)
                    nsq_ps = psum1.tile([P, 512], F32, tag="nsq", name="nsq")
                    for dt in range(2):
                        ps = psum.tile([P, 512], F32, tag="mm", name="mm")
                        for kt in range(8):
                            nc.tensor.matmul(
                                ps[:],
                                w_embed_sb[:, kt, dt * P : (dt + 1) * P],
                                xs[:, kt, loc],
                                start=(kt == 0),
                                stop=(kt == 7),
                            )
                        dst = embn_sb[dt][:, col0 : col0 + 512]
                        nc.scalar.activation(dst, ps[:], AF.Identity, bias=b_embed(dt))
                        sq = ph1s.tile([P, 512], F32, tag="sq", name="sq")
                        nc.scalar.activation(sq[:], dst, AF.Square)
                        nc.tensor.matmul(
                            nsq_ps[:], ones_sb[:], sq[:], start=(dt == 0), stop=(dt == 1)
                        )
                    # sqrt(sum sq), replicated across partitions by the ones-matmul
                    nc.scalar.activation(nrm[:, col0 : col0 + 512], nsq_ps[:], AF.Sqrt)

            # embedding output (own half) — before in-place normalization
            for dt in range(2):
                dma(o_embT[dt * P : (dt + 1) * P, :], embn_sb[dt][:, 0:HALF])

            nc.vector.tensor_scalar_add(nrm[:], nrm[:], 1e-8)
            nc.scalar.activation(den_own[:], nrm[:, 0:HALF], AF.Copy)
            nc.vector.reciprocal(nrm[:], nrm[:])
            for dt in range(2):
                nc.vector.tensor_mul(embn_sb[dt][:], embn_sb[dt][:], nrm[:])

        # ============ Phase 3b: K_flat (needs embn/den + coords reload) ========
        coo_r = o_cooT.rearrange("(ko p) t -> p ko t", p=P)
        with tc.tile_pool(name="ph3b", bufs=2) as ph3b:
            for c in range(N_CH):
                cs = slice(c * 512, (c + 1) * 512)
                cre = ph3b.tile([P, 2, 512], F32R, tag="cre", name="cre")
                dma(cre[:], coo_r[:, :, cs].bitcast(F32R))
                kfr = []
                for dt in range(2):
                    et = ph3b.tile([P, 512], F32R, tag=f"emb{dt}", name=f"emb{dt}")
                    nc.vector.tensor_mul(et[:], embn_sb[dt][:, cs], den_own[:, cs])
                    kfr.append(et)
                kfr += [cre[:, 0, :], cre[:, 1, :]]
                for nt in range(8):
                    ps = psum.tile([P, 512], F32, tag="mm", name="mm")
                    for kk in range(4):
                        nc.tensor.matmul(
                            ps[:],
                            w_kout_sb[:, kk, nt * P : (nt + 1) * P],
                            kfr[kk][:] if kk < 2 else kfr[kk],
                            start=(kk == 0),
                            stop=(kk == 3),
                        )
                    st = ph3b.tile([P, 512], F32, tag="kfst", name="kfst")
                    nc.scalar.activation(st[:], ps[:], AF.Identity, bias=b_kout(nt))
                    dma(o_kflT[nt * P : (nt + 1) * P, cs], st[:])

        # ================= Phase 2: sim + top-k =================
        N_QT = int(os.environ.get("KERNEL_N_QT", "16"))
        with tc.tile_pool(name="ph2", bufs=2) as ph2, tc.tile_pool(
            name="ph2b", bufs=2
        ) as ph2b:
            for qi in range(N_QT):
                q0 = qi * P
                sim = ph2.tile([P, S], F32, tag="sim", name="sim")
                for tc8 in range(8):
                    ps = psum.tile([P, 512], F32, tag="mm", name="mm")
                    for dt in range(2):
                        nc.tensor.matmul(
                            ps[:],
                            embn_sb[dt][:, q0 : q0 + P],
                            embn_sb[dt][:, tc8 * 512 : (tc8 + 1) * 512],
                            start=(dt == 0),
                            stop=(dt == 1),
                        )
                    nc.scalar.activation(
                        sim[:, tc8 * 512 : (tc8 + 1) * 512], ps[:], AF.Copy
                    )
                # mask self-similarity (query q0+p <-> key q0+p)
                nc.vector.scalar_tensor_tensor(
                    out=sim[:, q0 : q0 + P],
                    in0=ident_sb[:],
                    scalar=NEG,
                    in1=sim[:, q0 : q0 + P],
                    op0=OP.mult,
                    op1=OP.add,
                )
                # stage A: per-128-block top-8 values + local indices
                cand = ph2b.tile([P, 256], F32, tag="cand", name="cand")
                cloc = ph2b.tile([P, 256], U16, tag="cloc", name="cloc")
                for s in range(32):
                    nc.vector.max(
                        out=cand[:, s * 8 : (s + 1) * 8],
                        in_=sim[:, s * P : (s + 1) * P],
                    )
                    nc.vector.max_index(
                        out=cloc[:, s * 8 : (s + 1) * 8],
                        in_max=cand[:, s * 8 : (s + 1) * 8],
                        in_values=sim[:, s * P : (s + 1) * P],
                    )
                ci = ph2b.tile([P, 256], U16, tag="ci", name="ci")
                nc.vector.tensor_tensor(ci[:], cloc[:], segoff_sb[:], OP.add)
                # stage B: top-32 of the 256 candidates (sorted desc) + slots
                scores = score_acc[:, qi, :]
                slots = ph2b.tile([P, K], U16, tag="slots", name="slots")
                for rr in range(4):
                    nc.vector.max(out=scores[:, rr * 8 : (rr + 1) * 8], in_=cand[:])
                    nc.vector.max_index(
                        out=slots[:, rr * 8 : (rr + 1) * 8],
                        in_max=scores[:, rr * 8 : (rr + 1) * 8],
                        in_values=cand[:],
                    )
                    if rr < 3:
                        nc.vector.match_replace(
                            out=cand[:],
                            in_to_replace=scores[:, rr * 8 : (rr + 1) * 8],
                            in_values=cand[:],
                            imm_value=NEG,
                        )
                # indices: double local_scatter (per-partition gather of ci[slot])
                rank = ph2b.tile([P, 256], I16, tag="rank", name="rank")
                nc.gpsimd.local_scatter(
                    out_ap=rank[:],
                    data_ap=rankv_sb[:],
                    idxs_ap=slots[:].bitcast(I16),
                    channels=P,
                    num_elems=256,
                    num_idxs=K,
                )
                nc.gpsimd.tensor_scalar_add(rank[:], rank[:], -1)
                idxs = ph2b.tile([P, K], U16, tag="idxs", name="idxs")
                nc.gpsimd.local_scatter(
                    out_ap=idxs[:],
                    data_ap=ci[:],
                    idxs_ap=rank[:],
                    channels=P,
                    num_elems=K,
                    num_idxs=256,
                )
                nc.vector.tensor_copy(idx_acc[:, qi, :], idxs[:])
            dma(o_scores.rearrange("(qt p) k -> p qt k", p=P), score_acc[:])
            dma(o_idx.rearrange("(qt p) k -> p qt k", p=P), idx_acc[:])

        psum1.release()
        psum.release()
        pers.release()

    nc.compile()
    return nc


_NC_CACHE = None


def get_nc():
    global _NC_CACHE
    if _NC_CACHE is None:
        _NC_CACHE = build_nc()
    return _NC_CACHE


def make_in_maps(x, W_embed, b_embed, W_diag, W_chr, b_chr, W_ecc_proj, b_ecc_proj,
                 W_ecc1, b_ecc1, W_ecc2, b_ecc2, W_coord, b_coord, W_kout, b_kout):
    biases = np.zeros((P, 16), np.float32)
    biases[:, 0] = np.asarray(b_embed, np.float32)[0:P]
    biases[:, 1] = np.asarray(b_embed, np.float32)[P : 2 * P]
    biases[0:NCH, 2] = np.asarray(b_chr, np.float32)
    biases[0:ECC, 3] = np.asarray(b_ecc_proj, np.float32)
    biases[0 : 2 * ECC, 4] = np.asarray(b_ecc1, np.float32)
    biases[0:ECC, 5] = np.asarray(b_ecc2, np.float32)
    biases[:, 6] = np.asarray(b_coord, np.float32)[0:P]
    biases[:, 7] = np.asarray(b_coord, np.float32)[P : 2 * P]
    biases[:, 8:16] = np.asarray(b_kout, np.float32).reshape(8, P).T

    shared = {
        "w_embed": np.asarray(W_embed, np.float32),
        "w_diag": np.asarray(W_diag, np.float32),
        "w_chr": np.asarray(W_chr, np.float32),
        "w_eccp": np.asarray(W_ecc_proj, np.float32),
        "w_ecc1": np.asarray(W_ecc1, np.float32),
        "w_ecc2": np.asarray(W_ecc2, np.float32),
        "w_coord": np.asarray(W_coord, np.float32),
        "w_kout": np.asarray(W_kout, np.float32),
        "biases": biases,
        "ident": np.eye(P, dtype=np.float32),
        "segoff": np.broadcast_to(
            ((np.arange(256, dtype=np.uint16) >> 3) << 7), (P, 256)
        ).copy(),
        "rankv": np.broadcast_to(np.arange(1, K + 1, dtype=np.int16), (P, K)).copy(),
    }
    in_maps = []
    for c in range(8):
        b, h = c // 2, c % 2
        qoff = h * HALF
        xr = np.concatenate([x[b, qoff:], x[b, :qoff]], axis=0)  # [S, D]
        m = dict(shared)
        xrt = np.ascontiguousarray(xr.T)
        m["xt"] = xrt
        m["xtr"] = np.ascontiguousarray(xrt[:, :HALF])
        in_maps.append(m)
    return in_maps


def kernel(
    x,
    W_embed,
    b_embed,
    W_diag,
    W_chr,
    b_chr,
    W_ecc_proj,
    b_ecc_proj,
    W_ecc1,
    b_ecc1,
    W_ecc2,
    b_ecc2,
    W_coord,
    b_coord,
    W_kout,
    b_kout,
):
    global LAST_RESULTS, LAST_IN_MAPS
    x = np.asarray(x, np.float32)
    B = x.shape[0]
    assert (B, x.shape[1], x.shape[2]) == (4, S, D)

    in_maps = make_in_maps(
        x, W_embed, b_embed, W_diag, W_chr, b_chr, W_ecc_proj, b_ecc_proj,
        W_ecc1, b_ecc1, W_ecc2, b_ecc2, W_coord, b_coord, W_kout, b_kout,
    )
    LAST_IN_MAPS = in_maps
    nc = get_nc()
    trace = bool(int(os.environ.get("KERNEL_TRACE", "0")))
    res = run_bass_kernel_spmd(nc, in_maps, list(range(8)), trace=trace)
    LAST_RESULTS = res

    embedding = np.empty((B, S, DE), np.float32)
    metric = np.empty((B, S, D), np.float32)
    christoffel = np.empty((B, S, NCH), np.float32)
    knn_scores = np.empty((B, S, K), np.float32)
    knn_indices = np.empty((B, S, K), np.int32)
    ecc = np.empty((B, S, ECC), np.float32)
    coords = np.empty((B, S, DC), np.float32)
    K_flat = np.empty((B, S, D), np.float32)

    for c in range(8):
        b, h = c // 2, c % 2
        qoff = h * HALF
        sl = slice(qoff, qoff + HALF)
        o = res.results[c]
        embedding[b, sl] = o["o_embT"].T
        metric[b, sl] = o["o_metT"].T
        christoffel[b, sl] = o["o_chrT"].T
        knn_scores[b, sl] = o["o_scores"]
        knn_indices[b, sl] = (o["o_idx"].astype(np.int64) + qoff) % S
        ecc[b, sl] = o["o_eccT"].T
        coords[b, sl] = o["o_cooT"].T
        K_flat[b, sl] = o["o_kflT"].T

    min_heap = knn_scores[..., : K // 2].copy()
    max_heap = -knn_scores[..., K // 2 :]
    return (
        embedding,
        metric,
        christoffel,
        knn_scores,
        knn_indices.astype(np.int32),
        min_heap,
        max_heap,
        ecc,
        coords,
        K_flat,
    )
